# revision 10
# baseline (speedup 1.0000x reference)
"""Trainium2 Bass kernel for nn_MultiHeadAttention (B=4, T=2048, D=1024, H=16, hs=64).

Strategy (8 NeuronCores):
- Tensor-parallel over heads: core c computes QKV + RoPE + causal attention for
  heads 2c, 2c+1 (full batch), producing out^T chunk [128 d, 8192 tok].
- On-device AllToAll exchanges token-slices so core c holds out^T [1024 d, 1024 tok]
  for its 1/8 of tokens; it then does the output projection (+bias) for those rows.
- Host concatenates the 8 row-slices.

Numerics: fp32r (TF32-like, full PE rate at N>=256) for x/w_kq/scores/rope;
bf16 for attention weights, V, and the projection. All matmul accumulation fp32.

Layouts (all chosen so no on-device transposes are needed):
- host passes xT [D, B*T] (x transposed), w shards pre-transposed [D, 384] with
  RoPE even/odd rows pre-grouped, w_proj.T, plus constant cos/sin/mask tables.
- scores computed as S^T [ktok, qtok]; attention out as out^T [hs, qtok] with a
  ones-column in V producing the softmax row-sums for free.
"""

import numpy as np

B, T, D = 4, 2048, 1024
H, HS = 16, 64
W = 8               # cores
HPC = H // W        # heads per core
BT = B * T          # 8192
ROWS = BT // W      # tokens per core after exchange
P = 128
QC = T // 512       # 4 chunks of 512 tokens per batch
DC = D // P         # 8 contraction chunks
SCALE = 1.0 / 8.0
THETA = 10000.0

_CACHE = {}


def _build():
    import concourse.bass as bass
    import concourse.mybir as mybir
    import concourse.tile as tile
    from concourse import bacc

    f32 = mybir.dt.float32
    f32r = mybir.dt.float32r
    bf16 = mybir.dt.bfloat16
    Copy = mybir.ActivationFunctionType.Copy
    Exp = mybir.ActivationFunctionType.Exp
    mult = mybir.AluOpType.mult
    add = mybir.AluOpType.add

    nc = bacc.Bacc("TRN2", target_bir_lowering=False, debug=False, num_devices=W)

    xT = nc.dram_tensor("xT", [D, BT], f32, kind="ExternalInput").ap()
    wT = nc.dram_tensor("wT", [D, 3 * P], f32, kind="ExternalInput").ap()
    wpT = nc.dram_tensor("wpT", [D, D], f32, kind="ExternalInput").ap()
    bias = nc.dram_tensor("bias", [1, D], f32, kind="ExternalInput").ap()
    cosT = nc.dram_tensor("cosT", [P, T], f32, kind="ExternalInput").ap()
    sinT = nc.dram_tensor("sinT", [P, T], f32, kind="ExternalInput").ap()  # sign-baked
    maskT = nc.dram_tensor("maskT", [P, 896], f32, kind="ExternalInput").ap()
    y = nc.dram_tensor("y", [ROWS, D], f32, kind="ExternalOutput").ap()

    with tile.TileContext(nc) as tc:
        with (
            tc.tile_pool(name="const", bufs=1) as const,
            tc.tile_pool(name="qk", bufs=2) as qkp,
            tc.tile_pool(name="vp", bufs=2) as vp,
            tc.tile_pool(name="xload", bufs=2) as xload,
            tc.tile_pool(name="work", bufs=2) as work,
            tc.tile_pool(name="pt", bufs=17) as ptp,
            tc.tile_pool(name="outp", bufs=2) as outp,
            tc.tile_pool(name="ps_kq", bufs=2, space="PSUM") as ps_kq,
            tc.tile_pool(name="ps_v", bufs=1, space="PSUM") as ps_v,
            tc.tile_pool(name="ps_st", bufs=2, space="PSUM") as ps_st,
            tc.tile_pool(name="ps_ot", bufs=2, space="PSUM") as ps_ot,
            tc.tile_pool(name="ps_rep", bufs=1, space="PSUM") as ps_rep,
            tc.tile_pool(name="dram", bufs=1, space="DRAM") as dram,
        ):
            # ---------- constants / weights (staging pool closes early) ----------
            with tc.tile_pool(name="stage", bufs=1) as stage:
                wT_f = stage.tile([P, DC, 3 * P], f32)
                nc.sync.dma_start(wT_f[:], wT.rearrange("(o p) m -> p o m", p=P))
                wkq_r = const.tile([P, DC, 2 * P], f32r)
                nc.vector.tensor_copy(wkq_r[:], wT_f[:, :, 0:2 * P])
                wv_bf = const.tile([P, DC, P], bf16)
                nc.vector.tensor_copy(wv_bf[:], wT_f[:, :, 2 * P:3 * P])

                mask_f = stage.tile([P, 896], f32)
                nc.sync.dma_start(mask_f[:], maskT)
                mask_bf = const.tile([P, 896], bf16)
                nc.vector.tensor_copy(mask_bf[:], mask_f[:])

                bias_f = stage.tile([1, D], f32)
                nc.sync.dma_start(bias_f[:], bias)
                bias_bf = const.tile([1, D], bf16)
                nc.vector.tensor_copy(bias_bf[:], bias_f[:])

                ones_f = stage.tile([1, P], f32)
                nc.vector.memset(ones_f[:], 1.0)
                ones_bf = const.tile([1, P], bf16)
                nc.vector.tensor_copy(ones_bf[:], ones_f[:])
                ones_r = const.tile([1, HS], f32r)
                nc.vector.tensor_copy(ones_r[:], ones_f[:, 0:HS])

            cos_sb = const.tile([P, T], f32)
            sin_sb = const.tile([P, T], f32)
            nc.sync.dma_start(cos_sb[:], cosT)
            nc.sync.dma_start(sin_sb[:], sinT)

            a2a_in = dram.tile([W, P, ROWS], bf16)
            a2a_out = dram.tile([W, P, ROWS], bf16)

            # ---------- per-batch: QKV + RoPE, then attention ----------
            for b in range(B):
                qT_r = qkp.tile([P, T], f32r, tag="qT")
                kT_r = qkp.tile([P, T], f32r, tag="kT")
                v_sb = vp.tile([P, T // P, 2 * (HS + 1)], bf16, tag="v")
                nc.vector.memset(v_sb[:, :, HS:HS + 1], 1.0)
                nc.vector.memset(v_sb[:, :, 2 * HS + 1:2 * HS + 2], 1.0)

                # phase 1: 8 chunks of 256 tokens
                for ch in range(T // 256):
                    tb = ch * 256            # position within batch
                    t0 = b * T + tb          # global token offset
                    x_f = xload.tile([P, DC, 256], f32, tag="x_f", bufs=1)
                    nc.sync.dma_start(x_f[:], xT[:, t0:t0 + 256].rearrange("(o p) n -> p o n", p=P))
                    x_r = xload.tile([P, DC, 256], f32r, tag="x_r")
                    nc.scalar.activation(x_r[:], x_f[:], Copy)
                    x_bf = xload.tile([P, DC, 256], bf16, tag="x_bf")
                    nc.vector.tensor_copy(x_bf[:], x_f[:])

                    for part, dest in ((0, kT_r), (1, qT_r)):
                        psum = ps_kq.tile([P, 512], f32, tag="kq")
                        for dc in range(DC):
                            nc.tensor.matmul(
                                psum[:, 0:256], wkq_r[:, dc, part * P:(part + 1) * P],
                                x_r[:, dc], start=(dc == 0), stop=(dc == DC - 1),
                            )
                        # RoPE: rot = psum*cos + swap(psum)*sin_signed
                        tc_f = work.tile([P, 256], f32, tag="rope_c")
                        nc.vector.tensor_tensor(tc_f[:], psum[:, 0:256], cos_sb[:, tb:tb + 256], mult)
                        sw = work.tile([P, 256], f32, tag="rope_sw")
                        for hb in range(4):
                            b0 = hb * 32
                            nc.vector.tensor_copy(sw[b0 ^ 32:(b0 ^ 32) + 32, :], psum[b0:b0 + 32, 0:256])
                        nc.vector.tensor_tensor(sw[:], sw[:], sin_sb[:, tb:tb + 256], mult)
                        nc.vector.tensor_tensor(dest[:, tb:tb + 256], tc_f[:], sw[:], add)

                    # V (bf16): psum_v [tok 128, vcols 128]
                    for ts2 in range(2):
                        pv = ps_v.tile([P, P], f32, tag="v")
                        for dc in range(DC):
                            nc.tensor.matmul(
                                pv[:], x_bf[:, dc, ts2 * P:(ts2 + 1) * P], wv_bf[:, dc],
                                start=(dc == 0), stop=(dc == DC - 1),
                            )
                        lt = ch * 2 + ts2    # local token tile in batch
                        nc.scalar.activation(v_sb[:, lt, 0:HS], pv[:, 0:HS], Copy)
                        nc.scalar.activation(v_sb[:, lt, HS + 1:2 * HS + 1], pv[:, HS:2 * HS], Copy)

                # phase 2: attention for this batch
                for qc in range(QC):
                    nkt = 4 * qc + 4
                    for h in range(HPC):
                        hb = h * HS
                        pts = []
                        for kt in range(nkt):
                            pst = ps_st.tile([P, 512], f32, tag="st")
                            nc.tensor.matmul(
                                pst[:], kT_r[hb:hb + HS, kt * P:(kt + 1) * P],
                                qT_r[hb:hb + HS, qc * 512:(qc + 1) * 512],
                                start=True, stop=True,
                            )
                            pt = ptp.tile([P, 512], bf16, tag="pT")
                            nc.scalar.activation(pt[:], pst[:], Exp, scale=SCALE)
                            o = kt - 4 * qc
                            if o >= 0:
                                nc.vector.tensor_tensor(
                                    pt[:], pt[:], mask_bf[:, (3 - o) * P:(3 - o) * P + 512], mult,
                                )
                            pts.append(pt)
                        pot = ps_ot.tile([HS + 1, 512], f32, tag="ot")
                        c0 = h * (HS + 1)
                        for kt in range(nkt):
                            nc.tensor.matmul(
                                pot[:], v_sb[:, kt, c0:c0 + HS + 1], pts[kt][:],
                                start=(kt == 0), stop=(kt == nkt - 1),
                            )
                        # normalize: recip of row-sum row, replicate via rank-1 matmul
                        rec = work.tile([1, 512], f32r, tag="rec")
                        with nc.allow_low_precision(reason="f32r recip of softmax sums"):
                            nc.vector.reciprocal(rec[:], pot[HS:HS + 1, :])
                        prep = ps_rep.tile([HS, 512], f32, tag="rep")
                        nc.tensor.matmul(prep[:], ones_r[:], rec[:], start=True, stop=True)
                        rep_sb = work.tile([HS, 512], f32, tag="rep_sb")
                        nc.vector.tensor_copy(rep_sb[:], prep[:])
                        o_sb = outp.tile([HS, 512], bf16, tag="o_sb")
                        nc.vector.tensor_tensor(o_sb[:], pot[0:HS, :], rep_sb[:], mult)
                        # stage into A2A input bounce
                        g0 = b * T + qc * 512
                        nc.sync.dma_start(
                            a2a_in[g0 // ROWS, hb:hb + HS, g0 % ROWS:g0 % ROWS + 512], o_sb[:]
                        )

            # ---------- phase 3: AllToAll + projection ----------
            nc.gpsimd.collective_compute(
                "AllToAll", mybir.AluOpType.bypass,
                replica_groups=[list(range(W))],
                ins=[a2a_in[:]], outs=[a2a_out[:]],
            )
            wp_bf = const.tile([P, DC, D], bf16)
            for dc in range(DC):
                wp_f = work.tile([P, D], f32, tag="wp_f")
                nc.sync.dma_start(wp_f[:], wpT[dc * P:(dc + 1) * P, :])
                nc.vector.tensor_copy(wp_bf[:, dc], wp_f[:])

            for rt in range(ROWS // P):
                ot_bf = outp.tile([P, DC, P], bf16, tag="ot_bf")
                nc.sync.dma_start(ot_bf[:], a2a_out[:, :, rt * P:(rt + 1) * P].rearrange("o p n -> p o n"))
                for jc in range(2):
                    pp = ps_kq.tile([P, 512], f32, tag="kq")
                    for dc in range(DC):
                        nc.tensor.matmul(
                            pp[:], ot_bf[:, dc], wp_bf[:, dc, jc * 512:(jc + 1) * 512],
                            start=(dc == 0), stop=False,
                        )
                    nc.tensor.matmul(
                        pp[:], ones_bf[:], bias_bf[:, jc * 512:(jc + 1) * 512],
                        start=False, stop=True,
                    )
                    y_sb = outp.tile([P, 512], f32, tag="y_sb")
                    nc.vector.tensor_copy(y_sb[:], pp[:])
                    nc.sync.dma_start(y[rt * P:(rt + 1) * P, jc * 512:(jc + 1) * 512], y_sb[:])

    nc.compile()
    return nc


def _host_prep(x, w_kqv, w_proj, b_proj):
    xT = np.ascontiguousarray(x.reshape(BT, D).T)
    wpT = np.ascontiguousarray(w_proj.T)
    bias = np.ascontiguousarray(b_proj[None, :].astype(np.float32))

    # RoPE tables (position within batch), stacked to 128 partitions.
    m = np.arange(T, dtype=np.float64)
    i = np.arange(HS // 2, dtype=np.float64)
    theta = THETA ** (-2.0 * i / HS)
    ang = np.outer(theta, m)                      # [32, T]
    cos = np.cos(ang)
    sin = np.sin(ang)
    cosT = np.tile(cos, (4, 1)).astype(np.float32)         # [128, T]
    sin_sgn = np.concatenate([-sin, sin], axis=0)          # [64, T]
    sinT = np.tile(sin_sgn, (2, 1)).astype(np.float32)     # [128, T]

    # causal mask table M[r, cc] = 1 iff cc >= r + 384   -> slice (3-o)*128 gives
    # the diagonal-band mask: valid iff qcol >= krow + 128*o
    r = np.arange(P)[:, None]
    cc = np.arange(896)[None, :]
    maskT = (cc >= r + 384).astype(np.float32)

    perm = np.concatenate([np.arange(0, HS, 2), np.arange(1, HS, 2)])
    w_shards = []
    for c in range(W):
        rows = []
        for part in range(2):                    # k, q (with rope permutation)
            for h in range(HPC):
                base = part * D + (HPC * c + h) * HS
                rows.append(base + perm)
        for h in range(HPC):                     # v natural order
            base = 2 * D + (HPC * c + h) * HS
            rows.append(base + np.arange(HS))
        rows = np.concatenate(rows)
        w_shards.append(np.ascontiguousarray(w_kqv[rows].T))   # [D, 384]
    return xT, w_shards, wpT, bias, cosT, sinT, maskT


def kernel(x, w_kqv, w_proj, b_proj):
    from concourse import bass_utils

    x = np.asarray(x, dtype=np.float32)
    w_kqv = np.asarray(w_kqv, dtype=np.float32)
    w_proj = np.asarray(w_proj, dtype=np.float32)
    b_proj = np.asarray(b_proj, dtype=np.float32)

    if "nc" not in _CACHE:
        _CACHE["nc"] = _build()
    nc = _CACHE["nc"]

    xT, w_shards, wpT, bias, cosT, sinT, maskT = _host_prep(x, w_kqv, w_proj, b_proj)
    in_maps = [
        {
            "xT": xT, "wT": w_shards[c], "wpT": wpT, "bias": bias,
            "cosT": cosT, "sinT": sinT, "maskT": maskT,
        }
        for c in range(W)
    ]
    res = bass_utils.run_bass_kernel_spmd(nc, in_maps, core_ids=list(range(W)))
    out = np.concatenate([res.results[c]["y"] for c in range(W)], axis=0)
    return out.reshape(B, T, D)


# revision 13
# speedup vs baseline: 1.0120x; 1.0120x over previous
"""Trainium2 Bass kernel for nn_MultiHeadAttention (B=4, T=2048, D=1024, H=16, hs=64).

Strategy (8 NeuronCores):
- Tensor-parallel over heads: core c computes QKV + RoPE + causal attention for
  heads 2c, 2c+1 (full batch), producing out^T chunk [128 d, 8192 tok].
- On-device AllToAll exchanges token-slices so core c holds out^T [1024 d, 1024 tok]
  for its 1/8 of tokens; it then does the output projection (+bias) for those rows.
- Host concatenates the 8 row-slices.

Numerics: fp32r (TF32-like, full PE rate at N>=256) for x/w_kq/scores/rope;
bf16 for attention weights, V, and the projection. All matmul accumulation fp32.

Layouts (all chosen so no on-device transposes are needed):
- host passes xT [D, B*T] (x transposed), w shards pre-transposed [D, 384] with
  RoPE even/odd rows pre-grouped, w_proj.T, plus constant cos/sin/mask tables.
- scores computed as S^T [ktok, qtok]; attention out as out^T [hs, qtok] with a
  ones-column in V producing the softmax row-sums for free.
"""

import numpy as np

B, T, D = 4, 2048, 1024
H, HS = 16, 64
W = 8               # cores
HPC = H // W        # heads per core
BT = B * T          # 8192
ROWS = BT // W      # tokens per core after exchange
P = 128
QC = T // 512       # 4 chunks of 512 tokens per batch
DC = D // P         # 8 contraction chunks
SCALE = 1.0 / 8.0
THETA = 10000.0

_CACHE = {}


def _build(reps=1):
    import concourse.bass as bass
    import concourse.mybir as mybir
    import concourse.tile as tile
    from concourse import bacc
    from concourse.tile_rust import add_dep_helper

    f32 = mybir.dt.float32
    f32r = mybir.dt.float32r
    bf16 = mybir.dt.bfloat16
    Copy = mybir.ActivationFunctionType.Copy
    Exp = mybir.ActivationFunctionType.Exp
    mult = mybir.AluOpType.mult
    add = mybir.AluOpType.add

    nc = bacc.Bacc("TRN2", target_bir_lowering=False, debug=False, num_devices=W)

    xT = nc.dram_tensor("xT", [D, BT], f32, kind="ExternalInput").ap()
    wT = nc.dram_tensor("wT", [D, 3 * P], f32, kind="ExternalInput").ap()
    wpT = nc.dram_tensor("wpT", [D, D], f32, kind="ExternalInput").ap()
    bias = nc.dram_tensor("bias", [1, D], f32, kind="ExternalInput").ap()
    cosT = nc.dram_tensor("cosT", [P, T], f32, kind="ExternalInput").ap()
    sinT = nc.dram_tensor("sinT", [P, T], f32, kind="ExternalInput").ap()  # sign-baked
    maskT = nc.dram_tensor("maskT", [P, 896], f32, kind="ExternalInput").ap()
    y = nc.dram_tensor("y", [ROWS, D], f32, kind="ExternalOutput").ap()

    with tile.TileContext(nc) as tc:
        with (
            tc.tile_pool(name="const", bufs=1) as const,
            tc.tile_pool(name="qk", bufs=2) as qkp,
            tc.tile_pool(name="vp", bufs=2) as vp,
            tc.tile_pool(name="xload", bufs=2) as xload,
            tc.tile_pool(name="work", bufs=2) as work,
            tc.tile_pool(name="pt", bufs=17) as ptp,
            tc.tile_pool(name="outp", bufs=2) as outp,
            tc.tile_pool(name="ps_kq", bufs=2, space="PSUM") as ps_kq,
            tc.tile_pool(name="ps_v", bufs=1, space="PSUM") as ps_v,
            tc.tile_pool(name="ps_st", bufs=2, space="PSUM") as ps_st,
            tc.tile_pool(name="ps_ot", bufs=2, space="PSUM") as ps_ot,
            tc.tile_pool(name="ps_rep", bufs=1, space="PSUM") as ps_rep,
            tc.tile_pool(name="dram", bufs=1, space="DRAM") as dram,
        ):
            # ---------- constants / weights (staging pool closes early) ----------
            with tc.tile_pool(name="stage", bufs=1) as stage:
                wT_f = stage.tile([P, DC, 3 * P], f32)
                nc.sync.dma_start(wT_f[:], wT.rearrange("(o p) m -> p o m", p=P))
                wkq_r = const.tile([P, DC, 2 * P], f32r)
                nc.vector.tensor_copy(wkq_r[:], wT_f[:, :, 0:2 * P])
                wv_bf = const.tile([P, DC, P], bf16)
                nc.vector.tensor_copy(wv_bf[:], wT_f[:, :, 2 * P:3 * P])

                mask_f = stage.tile([P, 896], f32)
                nc.sync.dma_start(mask_f[:], maskT)
                mask_bf = const.tile([P, 896], bf16)
                nc.vector.tensor_copy(mask_bf[:], mask_f[:])

                bias_f = stage.tile([1, D], f32)
                nc.sync.dma_start(bias_f[:], bias)
                bias_bf = const.tile([1, D], bf16)
                nc.vector.tensor_copy(bias_bf[:], bias_f[:])

                ones_f = stage.tile([1, P], f32)
                nc.vector.memset(ones_f[:], 1.0)
                ones_bf = const.tile([1, P], bf16)
                nc.vector.tensor_copy(ones_bf[:], ones_f[:])
                ones_r = const.tile([1, HS], f32r)
                nc.vector.tensor_copy(ones_r[:], ones_f[:, 0:HS])

            cos_sb = const.tile([P, T], f32)
            sin_sb = const.tile([P, T], f32)
            nc.sync.dma_start(cos_sb[:], cosT)
            nc.sync.dma_start(sin_sb[:], sinT)

            wp_bf = const.tile([P, DC, D], bf16)
            for dc in range(DC):
                wp_f = work.tile([P, D], f32, tag="wp_f")
                nc.sync.dma_start(wp_f[:], wpT[dc * P:(dc + 1) * P, :])
                nc.vector.tensor_copy(wp_bf[:, dc], wp_f[:])

            a2a_in = dram.tile([W, P, ROWS], bf16)
            a2a_out = dram.tile([W, P, ROWS], bf16)

            prev_exits = None
            for _rep in range(reps):
              entries, exits = [], []
              # ---------- per-batch: QKV + RoPE, then attention ----------
              for b in range(B):
                qT_r = qkp.tile([P, T], f32r, tag="qT")
                kT_r = qkp.tile([P, T], f32r, tag="kT")
                v_sb = vp.tile([P, T // P, 2 * (HS + 1)], bf16, tag="v")
                entries.append(nc.vector.memset(v_sb[:, :, HS:HS + 1], 1.0))
                entries.append(nc.vector.memset(v_sb[:, :, 2 * HS + 1:2 * HS + 2], 1.0))

                # phase 1: 8 chunks of 256 tokens
                for ch in range(T // 256):
                    tb = ch * 256            # position within batch
                    t0 = b * T + tb          # global token offset
                    x_f = xload.tile([P, DC, 256], f32, tag="x_f", bufs=1)
                    entries.append(nc.sync.dma_start(
                        x_f[:], xT[:, t0:t0 + 256].rearrange("(o p) n -> p o n", p=P)))
                    x_r = xload.tile([P, DC, 256], f32r, tag="x_r")
                    nc.scalar.activation(x_r[:], x_f[:], Copy)
                    x_bf = xload.tile([P, DC, 256], bf16, tag="x_bf")
                    nc.vector.tensor_copy(x_bf[:], x_f[:])

                    for part, dest in ((0, kT_r), (1, qT_r)):
                        psum = ps_kq.tile([P, 512], f32, tag="kq")
                        for dc in range(DC):
                            nc.tensor.matmul(
                                psum[:, 0:256], wkq_r[:, dc, part * P:(part + 1) * P],
                                x_r[:, dc], start=(dc == 0), stop=(dc == DC - 1),
                            )
                        # RoPE: rot = psum*cos + swap(psum)*sin_signed
                        tc_f = work.tile([P, 256], f32, tag="rope_c")
                        nc.vector.tensor_tensor(tc_f[:], psum[:, 0:256], cos_sb[:, tb:tb + 256], mult)
                        sw = work.tile([P, 256], f32, tag="rope_sw")
                        for hb in range(4):
                            b0 = hb * 32
                            nc.vector.tensor_copy(sw[b0 ^ 32:(b0 ^ 32) + 32, :], psum[b0:b0 + 32, 0:256])
                        nc.vector.tensor_tensor(sw[:], sw[:], sin_sb[:, tb:tb + 256], mult)
                        nc.vector.tensor_tensor(dest[:, tb:tb + 256], tc_f[:], sw[:], add)

                    # V (bf16): psum_v [tok 128, vcols 128]
                    for ts2 in range(2):
                        pv = ps_v.tile([P, P], f32, tag="v")
                        for dc in range(DC):
                            nc.tensor.matmul(
                                pv[:], x_bf[:, dc, ts2 * P:(ts2 + 1) * P], wv_bf[:, dc],
                                start=(dc == 0), stop=(dc == DC - 1),
                            )
                        lt = ch * 2 + ts2    # local token tile in batch
                        nc.scalar.activation(v_sb[:, lt, 0:HS], pv[:, 0:HS], Copy)
                        nc.scalar.activation(v_sb[:, lt, HS + 1:2 * HS + 1], pv[:, HS:2 * HS], Copy)

                # phase 2: attention for this batch
                for qc in range(QC):
                    nkt = 4 * qc + 4
                    for h in range(HPC):
                        hb = h * HS
                        pts = []
                        for kt in range(nkt):
                            pst = ps_st.tile([P, 512], f32, tag="st")
                            nc.tensor.matmul(
                                pst[:], kT_r[hb:hb + HS, kt * P:(kt + 1) * P],
                                qT_r[hb:hb + HS, qc * 512:(qc + 1) * 512],
                                start=True, stop=True,
                            )
                            pt = ptp.tile([P, 512], bf16, tag="pT")
                            nc.scalar.activation(pt[:], pst[:], Exp, scale=SCALE)
                            o = kt - 4 * qc
                            if o >= 0:
                                nc.vector.tensor_tensor(
                                    pt[:], pt[:], mask_bf[:, (3 - o) * P:(3 - o) * P + 512], mult,
                                )
                            pts.append(pt)
                        pot = ps_ot.tile([HS + 1, 512], f32, tag="ot")
                        c0 = h * (HS + 1)
                        for kt in range(nkt):
                            nc.tensor.matmul(
                                pot[:], v_sb[:, kt, c0:c0 + HS + 1], pts[kt][:],
                                start=(kt == 0), stop=(kt == nkt - 1),
                            )
                        # normalize: recip of row-sum row, replicate via rank-1 matmul
                        rec = work.tile([1, 512], f32r, tag="rec")
                        with nc.allow_low_precision(reason="f32r recip of softmax sums"):
                            nc.vector.reciprocal(rec[:], pot[HS:HS + 1, :])
                        prep = ps_rep.tile([HS, 512], f32, tag="rep")
                        nc.tensor.matmul(prep[:], ones_r[:], rec[:], start=True, stop=True)
                        rep_sb = work.tile([HS, 512], f32, tag="rep_sb")
                        nc.vector.tensor_copy(rep_sb[:], prep[:])
                        o_sb = outp.tile([HS, 512], bf16, tag="o_sb")
                        nc.vector.tensor_tensor(o_sb[:], pot[0:HS, :], rep_sb[:], mult)
                        # stage into A2A input bounce
                        g0 = b * T + qc * 512
                        nc.sync.dma_start(
                            a2a_in[g0 // ROWS, hb:hb + HS, g0 % ROWS:g0 % ROWS + 512], o_sb[:]
                        )

              # ---------- phase 3: AllToAll + projection ----------
              nc.gpsimd.collective_compute(
                  "AllToAll", mybir.AluOpType.bypass,
                  replica_groups=[list(range(W))],
                  ins=[a2a_in[:]], outs=[a2a_out[:]],
              )
              for rt in range(ROWS // P):
                  ot_bf = outp.tile([P, DC, P], bf16, tag="ot_bf")
                  nc.sync.dma_start(ot_bf[:], a2a_out[:, :, rt * P:(rt + 1) * P].rearrange("o p n -> p o n"))
                  for jc in range(2):
                      pp = ps_kq.tile([P, 512], f32, tag="kq")
                      for dc in range(DC):
                          nc.tensor.matmul(
                              pp[:], ot_bf[:, dc], wp_bf[:, dc, jc * 512:(jc + 1) * 512],
                              start=(dc == 0), stop=False,
                          )
                      nc.tensor.matmul(
                          pp[:], ones_bf[:], bias_bf[:, jc * 512:(jc + 1) * 512],
                          start=False, stop=True,
                      )
                      y_sb = outp.tile([P, 512], f32, tag="y_sb")
                      nc.vector.tensor_copy(y_sb[:], pp[:])
                      exits.append(nc.sync.dma_start(
                          y[rt * P:(rt + 1) * P, jc * 512:(jc + 1) * 512], y_sb[:]))

              if prev_exits is not None:
                  for xo in prev_exits:
                      for en in entries:
                          add_dep_helper(xo.ins, en.ins, sync=True, reason="rep chain")
              prev_exits = exits

    nc.compile()
    return nc


def _host_prep(x, w_kqv, w_proj, b_proj):
    xT = np.ascontiguousarray(x.reshape(BT, D).T)
    wpT = np.ascontiguousarray(w_proj.T)
    bias = np.ascontiguousarray(b_proj[None, :].astype(np.float32))

    # RoPE tables (position within batch), stacked to 128 partitions.
    m = np.arange(T, dtype=np.float64)
    i = np.arange(HS // 2, dtype=np.float64)
    theta = THETA ** (-2.0 * i / HS)
    ang = np.outer(theta, m)                      # [32, T]
    cos = np.cos(ang)
    sin = np.sin(ang)
    cosT = np.tile(cos, (4, 1)).astype(np.float32)         # [128, T]
    sin_sgn = np.concatenate([-sin, sin], axis=0)          # [64, T]
    sinT = np.tile(sin_sgn, (2, 1)).astype(np.float32)     # [128, T]

    # causal mask table M[r, cc] = 1 iff cc >= r + 384   -> slice (3-o)*128 gives
    # the diagonal-band mask: valid iff qcol >= krow + 128*o
    r = np.arange(P)[:, None]
    cc = np.arange(896)[None, :]
    maskT = (cc >= r + 384).astype(np.float32)

    perm = np.concatenate([np.arange(0, HS, 2), np.arange(1, HS, 2)])
    w_shards = []
    for c in range(W):
        rows = []
        for part in range(2):                    # k, q (with rope permutation)
            for h in range(HPC):
                base = part * D + (HPC * c + h) * HS
                rows.append(base + perm)
        for h in range(HPC):                     # v natural order
            base = 2 * D + (HPC * c + h) * HS
            rows.append(base + np.arange(HS))
        rows = np.concatenate(rows)
        w_shards.append(np.ascontiguousarray(w_kqv[rows].T))   # [D, 384]
    return xT, w_shards, wpT, bias, cosT, sinT, maskT


def kernel(x, w_kqv, w_proj, b_proj):
    from concourse import bass_utils

    x = np.asarray(x, dtype=np.float32)
    w_kqv = np.asarray(w_kqv, dtype=np.float32)
    w_proj = np.asarray(w_proj, dtype=np.float32)
    b_proj = np.asarray(b_proj, dtype=np.float32)

    if "nc" not in _CACHE:
        _CACHE["nc"] = _build()
    nc = _CACHE["nc"]

    xT, w_shards, wpT, bias, cosT, sinT, maskT = _host_prep(x, w_kqv, w_proj, b_proj)
    in_maps = [
        {
            "xT": xT, "wT": w_shards[c], "wpT": wpT, "bias": bias,
            "cosT": cosT, "sinT": sinT, "maskT": maskT,
        }
        for c in range(W)
    ]
    res = bass_utils.run_bass_kernel_spmd(nc, in_maps, core_ids=list(range(W)))
    out = np.concatenate([res.results[c]["y"] for c in range(W)], axis=0)
    return out.reshape(B, T, D)
